# revision 4
# baseline (speedup 1.0000x reference)
"""Trainium2 Bass kernel for nn_ABNet_U (multi-branch MLP + CBF-QP head).

Data-parallel over batch: 16384 rows -> 8 NeuronCores x 2048 rows, weights
replicated and host-prepped into K-major layouts.  The three large middle
GEMMs (L2/L3/L4) run in fp8-e4m3 with DoubleRow perf mode; L1 and the
small heads stay bf16.  GEMMs accumulate in fp32 PSUM with fused
scale+bias+activation eviction on the Scalar/Vector engines.

Schedule notes (all measured on HW traces):
- L1 runs as true-K=4 matmuls packed 4-at-a-time into the PE's 32-row
  tile groups; its inputs are tiny ([4,2048]) and are DMA'd once per row
  group from four different engine queues to dodge the ~0.7us/post cost.
- h1's eviction wall (~14us over 2 engines) is absorbed by k-interleaving
  the first two L2 m-tiles so their matmuls pace with the eviction stream.
- The trig/barrier geometry for the QP tail is emitted AFTER the L2 pass
  so the in-order DVE queue keeps L1 group-1 evictions ahead of it.
- Two PSUM pools: 4 banks for the main GEMM stream, 4 for L1/transposes/
  L5 so neither ring blocks the other.
- The L5/transpose/QP tail is pipelined in four batch quarters (QP
  elementwise chains alternate DVE/GpSimd); only the last chain is
  exposed past the matmul stream.
Set FP8_LAYERS = frozenset() for the bf16-exact fallback.
"""

import sys

sys.path.insert(0, "/opt/trn_rl_repo")

import numpy as np
import ml_dtypes

import concourse.bass as bass
import concourse.mybir as mybir
import concourse.tile as tile
from concourse import bacc
from concourse.bass_utils import run_bass_kernel_spmd
from concourse.masks import make_identity

N_CORES = 8
B_GLOBAL = 16384
B = B_GLOBAL // N_CORES  # 2048 rows per core
P = 128
CH = B // P              # 16 batch chunks of 128 (tail layout)
NF = 512                 # matmul free-dim chunk
NB = B // NF             # 4 free chunks
HEADS = 10

AF = mybir.ActivationFunctionType
ALU = mybir.AluOpType
AX = mybir.AxisListType
F32 = mybir.dt.float32
BF16 = mybir.dt.bfloat16
I32 = mybir.dt.int32

TWO_PI = float(2.0 * np.pi)
HALF_PI = float(0.5 * np.pi)

FP8_LAYERS = frozenset({"l2", "l3", "l4"})  # override via configure()
FP8 = mybir.dt.float8e4

_CACHED_NC = None


def _build(fp8_layers=frozenset()):
    nc = bacc.Bacc(
        "TRN2",
        target_bir_lowering=False,
        debug=False,
        enable_asserts=False,
        num_devices=N_CORES,
    )

    def din(name, shape, dt=F32):
        return nc.dram_tensor(name, list(shape), dt, kind="ExternalInput").ap()

    xt4 = din("xt4", (16, B), BF16)          # x.T replicated in 4 row groups
    xn = din("xn", (P, CH, 4))               # x shard, [p, chunk, feat] fp32
    w14 = din("w14", (16, 2048), BF16)       # W1.T replicated in 4 row groups
    w2 = din("w2", (P, 16, 16, P), FP8 if "l2" in fp8_layers else BF16)
    w3 = din("w3", (P, 16, 16, P), FP8 if "l3" in fp8_layers else BF16)
    w41 = din("w41", (P, 8, 8, P), FP8 if "l4" in fp8_layers else BF16)
    w42 = din("w42", (P, 8, 8, P), FP8 if "l4" in fp8_layers else BF16)
    esc2 = din("esc2", (P, 1))
    esc3 = din("esc3", (P, 1))
    esc4 = din("esc4", (P, 1))
    w51 = din("w51", (P, 8, 20), BF16)       # [p, kt, m]
    w52 = din("w52", (P, 8, 11), BF16)
    b1 = din("b1", (P, 16))
    b2 = din("b2", (P, 16))
    b3 = din("b3", (P, 16))
    b41 = din("b41", (P, 8))
    b42 = din("b42", (P, 8))
    b51 = din("b51", (20,))
    b52 = din("b52", (11,))
    stdb = din("stdb", (P, 4))
    meanb = din("meanb", (P, 4))
    mlb = din("mlb", (P, 2))
    islb = din("islb", (P, 2))
    wtv = din("wtv", (10,))
    out = nc.dram_tensor("out", [P, CH, 2], F32, kind="ExternalOutput").ap()

    with tile.TileContext(nc) as tc:
        from contextlib import ExitStack

        with ExitStack() as ctx:
            const = ctx.enter_context(tc.tile_pool(name="const", bufs=1))
            wpool = ctx.enter_context(tc.tile_pool(name="wpool", bufs=3))
            w4pool = ctx.enter_context(tc.tile_pool(name="w4pool", bufs=1))
            hpool = ctx.enter_context(tc.tile_pool(name="hpool", bufs=2))
            psumB = ctx.enter_context(tc.tile_pool(name="psumB", bufs=2, space="PSUM"))
            psumA = ctx.enter_context(tc.tile_pool(name="psumA", bufs=2, space="PSUM"))
            tp = ctx.enter_context(tc.tile_pool(name="tp", bufs=1))

            # ---- L1-critical loads.  Row-group replicas of x.T / W1.T go
            # out on four different engine queues (posting is ~0.7us/DMA
            # and strictly serial per queue).
            xtb = const.tile([P, B], BF16, tag="xtb")
            w1tb = const.tile([P, 2048], BF16, tag="w1tb")
            # group 0 (partitions 0..3) gates the first L1 matmul: its two
            # posts go first on the sync queue.  Groups 1-3 feed only the
            # L1 group-1 quads (~20us in) and ride the scalar/gpsimd queues
            # (posting is ~0.7us/DMA and strictly serial per queue).
            nc.sync.dma_start(xtb[0:4, :], xt4[0:4, :])
            nc.sync.dma_start(w1tb[0:4, :], w14[0:4, :])
            qeng = [None, nc.scalar, nc.gpsimd, nc.gpsimd]
            for i in range(1, 4):
                qeng[i].dma_start(xtb[32 * i : 32 * i + 4, :], xt4[4 * i : 4 * i + 4, :])
                qeng[i].dma_start(w1tb[32 * i : 32 * i + 4, :], w14[4 * i : 4 * i + 4, :])
            b1t = const.tile([P, 16], F32, tag="b1")
            nc.sync.dma_start(b1t[:], b1)
            wcol2_0 = wpool.tile(
                [P, 16, P], FP8 if "l2" in fp8_layers else BF16,
                tag="wcol", name="wcol2_0",
            )
            nc.sync.dma_start(wcol2_0[:], w2[:, 0])
            wcol2_1 = wpool.tile(
                [P, 16, P], FP8 if "l2" in fp8_layers else BF16,
                tag="wcol", name="wcol2_1",
            )
            nc.sync.dma_start(wcol2_1[:], w2[:, 1])
            b2t = const.tile([P, 16], F32, tag="b2")
            nc.sync.dma_start(b2t[:], b2)
            esc2t = const.tile([P, 1], F32, tag="esc2t")
            nc.sync.dma_start(esc2t[:], esc2)
            w51c = const.tile([P, 8, 20], BF16, tag="w51c")
            nc.gpsimd.dma_start(w51c[:], w51)
            w52c = const.tile([P, 8, 11], BF16, tag="w52c")
            nc.gpsimd.dma_start(w52c[:], w52)

            # ACT table warms: the Tile scheduler floats these zero-dep ops
            # to the front of the ACT queue, so they all land in the DMA
            # window before the first L1 eviction needs Relu.
            tbl = const.tile([1, 2], F32, tag="tbl")
            nc.vector.memset(tbl[:], 0.25)
            nc.scalar.activation(tbl[:, 1:2], tbl[:, 0:1], AF.Relu)
            nc.scalar.activation(tbl[:, 1:2], tbl[:, 0:1], AF.Sin)
            nc.scalar.activation(tbl[:, 1:2], tbl[:, 0:1], AF.Sigmoid)
            # (Exp/Identity first-uses pay their own table loads in GEMM
            # slack; warming them here would spill past the DMA window)

            # PE warm-up: dummy matmuls fill the idle window while the L1
            # input DMAs land, and push the HAM clock ramp to full speed
            # before the real matmul stream begins.
            wrm = const.tile([P, NF], BF16, tag="wrm")
            nc.vector.memset(wrm[:], 0.0)
            wps = psumA.tile([P, 2 * NF], F32, tag="mm", name="wps")
            for _ in range(16):
                nc.tensor.matmul(
                    wps[:, :NF], wrm[:, :P], wrm[:], start=True, stop=True
                )

            # ---- L1: h1 = relu(W1 @ x^T + b1), true K=4, 4 m-tiles packed
            # into the PE's four 32-row tile groups (concurrent matmuls).
            # Evictions bound this layer: strict ACT/DVE alternation.
            h1dt = FP8 if "l2" in fp8_layers else BF16
            h2dt = FP8 if "l3" in fp8_layers else BF16
            h3dt = FP8 if "l4" in fp8_layers else BF16
            h1 = hpool.tile([P, 16, B], h1dt, tag="act", name="h1")

            def emit_l1_quad(mq, n):
                # 4 m-tiles (4mq+i) for one 512-col n chunk; 2 psum tiles
                pss = [
                    psumA.tile([P, 2 * NF], F32, tag="mm", name="l1q")
                    for _ in range(2)
                ]
                for i in range(4):
                    m = 4 * mq + i
                    nc.tensor.matmul(
                        pss[i // 2][:, (i % 2) * NF : (i % 2) * NF + NF],
                        w1tb[32 * i : 32 * i + 4, m * P : (m + 1) * P],
                        xtb[32 * i : 32 * i + 4, n * NF : (n + 1) * NF],
                        start=True,
                        stop=True,
                        tile_position=(32 * i, 0),
                    )
                for i in range(4):
                    m = 4 * mq + i
                    dst = h1[:, m, n * NF : (n + 1) * NF]
                    src = pss[i // 2][:, (i % 2) * NF : (i % 2) * NF + NF]
                    if i % 2 == 0:
                        nc.scalar.activation(
                            dst, src, AF.Relu, bias=b1t[:, m : m + 1]
                        )
                    else:
                        nc.vector.tensor_scalar(
                            dst, src, b1t[:, m : m + 1], 0.0,
                            op0=ALU.add, op1=ALU.max,
                        )

            # L1 is emitted entirely as 4-way tile-packed quads (4 m-tiles
            # concurrent in the PE's 32-row tile groups).  The phase wall
            # is the h1 eviction stream on ACT/DVE; quads + the m0/m1 L2
            # k2 steps below keep the PE dense while it drains.

            # ---- generic streamed GEMM layer ----
            def mlp_layer(wdram, KT, MT, MD, hin, kin_base, btile, evict,
                          prefetched=None, dr=False, groups=None):
                wdt = FP8 if dr else BF16
                if groups is None:
                    groups = range(NB // 2)
                for m in range(MT):
                    mp = min(P, MD - m * P)
                    if prefetched is not None and m in prefetched:
                        wcol = prefetched[m]
                    else:
                        wcol = wpool.tile([P, KT, mp], wdt, tag="wcol")
                        if len(wdram.shape) == 4:
                            nc.sync.dma_start(wcol[:], wdram[:, m])
                        else:
                            nc.sync.dma_start(wcol[:], wdram)
                    for g in groups:
                        ps = psumB.tile([P, 2 * NF], F32, tag="mm")
                        for half in range(2):
                            n = 2 * g + half
                            if dr:
                                for k2 in range(KT // 2):
                                    nc.tensor.matmul(
                                        ps[:mp, half * NF : (half + 1) * NF],
                                        wcol[:, 2 * k2 : 2 * k2 + 2, :],
                                        hin[:, kin_base + 2 * k2 : kin_base + 2 * k2 + 2,
                                            n * NF : (n + 1) * NF],
                                        start=(k2 == 0),
                                        stop=(k2 == KT // 2 - 1),
                                        perf_mode=mybir.MatmulPerfMode.DoubleRow,
                                    )
                            else:
                                for k in range(KT):
                                    nc.tensor.matmul(
                                        ps[:mp, half * NF : (half + 1) * NF],
                                        wcol[:, k, :],
                                        hin[:, kin_base + k, n * NF : (n + 1) * NF],
                                        start=(k == 0),
                                        stop=(k == KT - 1),
                                    )
                        evict(m, g, ps[:mp])

            # ---- L2 ----
            h2 = hpool.tile([P, 16, B], h2dt, tag="act", name="h2")

            def ev_h(hout, btile, m_off=0, scale=1.0, alt=False):
                def _e(m, g, ps):
                    dst = hout[:, m_off + m, 2 * g * NF : 2 * (g + 1) * NF]
                    if alt and m in (0, 2, 4):
                        # scale-free path (L4 scale is folded into b4x/w5x
                        # on the host): relu(ps + b).  The last m stays on
                        # ACT so L5's trailing k-pass isn't DVE-gated.
                        nc.vector.tensor_scalar(
                            dst, ps, btile[:, m : m + 1], 0.0,
                            op0=ALU.add, op1=ALU.max,
                        )
                    else:
                        nc.scalar.activation(
                            dst, ps, AF.Relu, bias=btile[:, m : m + 1], scale=scale,
                        )
                return _e

            sc2 = esc2t[:]
            ev2 = ev_h(h2, b2t, 0, sc2)
            l2dr = "l2" in fp8_layers
            w2dt = FP8 if l2dr else BF16

            def l2_passes(ms, g, wcols, pss):
                # interleave the k2 loops of several m-tiles so early-m
                # stalls on streaming h1 evictions are absorbed by peers
                for k2 in range(8):
                    for j, m in enumerate(ms):
                        for half in range(2):
                            n = 2 * g + half
                            if l2dr:
                                nc.tensor.matmul(
                                    pss[j][:, half * NF : (half + 1) * NF],
                                    wcols[j][:, 2 * k2 : 2 * k2 + 2, :],
                                    h1[:, 2 * k2 : 2 * k2 + 2, n * NF : (n + 1) * NF],
                                    start=(k2 == 0),
                                    stop=(k2 == 7),
                                    perf_mode=mybir.MatmulPerfMode.DoubleRow,
                                )
                            else:
                                for kk in range(2):
                                    k = 2 * k2 + kk
                                    nc.tensor.matmul(
                                        pss[j][:, half * NF : (half + 1) * NF],
                                        wcols[j][:, k, :],
                                        h1[:, k, n * NF : (n + 1) * NF],
                                        start=(k == 0),
                                        stop=(k == 15),
                                    )
                for j, m in enumerate(ms):
                    ev2(m, g, pss[j][:])

            def emit_l2_group(m, g, wcol):
                ps = psumB.tile([P, 2 * NF], F32, tag="mm", name="l2ps")
                l2_passes([m], g, [wcol], [ps])

            # Pass A: the m0/m1 k2 stream is windowed between L1 quads so
            # the PE chases the h1 eviction stream (quad mq covers h1
            # k-slices 4mq..4mq+3 = k2 steps 2mq, 2mq+1).
            psA = psumB.tile([P, 2 * NF], F32, tag="mm", name="l2pA")
            psB = psumB.tile([P, 2 * NF], F32, tag="mm", name="l2pB")
            wcA = [wcol2_0, wcol2_1]
            psAB = [psA, psB]

            def passA_k2(k2):
                for j in range(2):
                    for half in range(2):
                        if l2dr:
                            nc.tensor.matmul(
                                psAB[j][:, half * NF : (half + 1) * NF],
                                wcA[j][:, 2 * k2 : 2 * k2 + 2, :],
                                h1[:, 2 * k2 : 2 * k2 + 2, half * NF : (half + 1) * NF],
                                start=(k2 == 0),
                                stop=(k2 == 7),
                                perf_mode=mybir.MatmulPerfMode.DoubleRow,
                            )
                        else:
                            for kk in range(2):
                                k = 2 * k2 + kk
                                nc.tensor.matmul(
                                    psAB[j][:, half * NF : (half + 1) * NF],
                                    wcA[j][:, k, :],
                                    h1[:, k, half * NF : (half + 1) * NF],
                                    start=(k == 0),
                                    stop=(k == 15),
                                )

            emit_l1_quad(0, 0)
            emit_l1_quad(0, 1)
            emit_l1_quad(1, 0)
            emit_l1_quad(1, 1)
            passA_k2(0)
            passA_k2(1)
            emit_l1_quad(2, 0)
            emit_l1_quad(2, 1)
            passA_k2(2)
            passA_k2(3)
            emit_l1_quad(3, 0)
            emit_l1_quad(3, 1)
            passA_k2(4)
            passA_k2(5)
            emit_l1_quad(0, 2)
            emit_l1_quad(0, 3)
            passA_k2(6)
            passA_k2(7)
            ev2(0, 0, psA[:])
            ev2(1, 0, psB[:])
            emit_l1_quad(1, 2)
            emit_l1_quad(1, 3)
            for mm_ in (2, 3):
                wcol = wpool.tile([P, 16, P], w2dt, tag="wcol", name="w2a")
                nc.sync.dma_start(wcol[:], w2[:, mm_])
                emit_l2_group(mm_, 0, wcol)
            emit_l1_quad(2, 2)
            emit_l1_quad(2, 3)
            for mm_ in (4, 5):
                wcol = wpool.tile([P, 16, P], w2dt, tag="wcol", name="w2a")
                nc.sync.dma_start(wcol[:], w2[:, mm_])
                emit_l2_group(mm_, 0, wcol)
            emit_l1_quad(3, 2)
            emit_l1_quad(3, 3)
            for mm_ in range(6, 16):
                wcol = wpool.tile([P, 16, P], w2dt, tag="wcol", name="w2a")
                nc.sync.dma_start(wcol[:], w2[:, mm_])
                emit_l2_group(mm_, 0, wcol)
            # Pass B: second column-group (weights re-streamed; traffic hides).
            for m in range(16):
                wcol = wpool.tile([P, 16, P], w2dt, tag="wcol", name="w2b")
                nc.sync.dma_start(wcol[:], w2[:, m])
                emit_l2_group(m, 1, wcol)

            # ---- remaining constants + tail geometry, emitted after the
            # L2 stream so the in-order DVE/Sync queues never block it.
            b3t = const.tile([P, 16], F32, tag="b3")
            nc.sync.dma_start(b3t[:], b3)
            b41t = const.tile([P, 8], F32, tag="b41")
            nc.sync.dma_start(b41t[:], b41)
            b42t = const.tile([P, 8], F32, tag="b42")
            nc.sync.dma_start(b42t[:], b42)
            b51t = const.tile([20, 1], F32, tag="b51")
            nc.sync.dma_start(b51t[:], b51[:, None])
            b52t = const.tile([11, 1], F32, tag="b52")
            nc.sync.dma_start(b52t[:], b52[:, None])
            stdt = const.tile([P, 4], F32, tag="stdt")
            nc.sync.dma_start(stdt[:], stdb)
            meant = const.tile([P, 4], F32, tag="meant")
            nc.sync.dma_start(meant[:], meanb)
            mlt = const.tile([P, 2], F32, tag="mlt")
            nc.sync.dma_start(mlt[:], mlb)
            islt = const.tile([P, 2], F32, tag="islt")
            nc.sync.dma_start(islt[:], islb)
            esc3t = const.tile([P, 1], F32, tag="esc3t")
            nc.sync.dma_start(esc3t[:], esc3)
            halfpi = const.tile([P, 1], F32, tag="halfpi")
            nc.vector.memset(halfpi[:], HALF_PI)
            ident = const.tile([P, P], F32)
            make_identity(nc, ident[:])

            # softmax(wt) DVE chain (PE broadcast deferred until after L4)
            wtt = const.tile([1, 10], F32, tag="wtt")
            nc.sync.dma_start(wtt[:], wtv[None, :])
            mx = const.tile([1, 1], F32, tag="mx")
            nc.vector.reduce_max(mx[:, 0:1], wtt[:], axis=AX.X)
            nm = const.tile([1, 1], F32, tag="nm")
            nc.vector.tensor_scalar_mul(nm[:], mx[:], -1.0)
            ex = const.tile([1, 10], F32, tag="ex")
            nc.scalar.activation(ex[:], wtt[:], AF.Exp, bias=nm[:])
            sm = const.tile([1, 1], F32, tag="sm")
            nc.vector.reduce_sum(sm[:, 0:1], ex[:], axis=AX.X)
            inv = const.tile([1, 1], F32, tag="inv")
            nc.vector.reciprocal(inv[:], sm[:])
            wv10 = const.tile([1, 10], F32, tag="wv10")
            nc.vector.tensor_scalar_mul(wv10[:], ex[:], inv[:])
            wvp = const.tile([32, 32], F32, tag="wvp")
            nc.vector.memset(wvp[:], 0.0)
            nc.vector.tensor_copy(
                wvp[0:1, 0:20].rearrange("p (h c) -> p h c", c=2),
                wv10[:, :, None].to_broadcast([1, 10, 2]),
            )
            onesp = const.tile([32, P], F32, tag="onesp")
            nc.vector.memset(onesp[:], 0.0)
            nc.vector.memset(onesp[0:1, :], 1.0)

            # combined x51/x52 head tile (rows 0..19 = x51, 32..42 = x52;
            # rows 20..31 / 43..63 are never read downstream -> no memset)
            x5cat = tp.tile([64, B], F32, tag="x5cat")

            # ---- tail part 1: geometry from x only — computed on the DVE
            # underneath the L3/L4 GEMM phases.
            def t3(tag, d=1):
                return tp.tile([P, CH, d], F32, tag=tag, name=tag)

            xnt = t3("xnt", 4)
            nc.sync.dma_start(xnt[:], xn)
            x0 = t3("x0", 4)
            nc.vector.tensor_mul(
                x0[:], xnt[:], stdt[:, None, :].to_broadcast([P, CH, 4])
            )
            nc.vector.tensor_add(
                x0[:], x0[:], meant[:, None, :].to_broadcast([P, CH, 4])
            )

            th = x0[:, :, 0::2]   # [P, CH, 2] angles
            wv_ = x0[:, :, 1::2]  # [P, CH, 2] angular velocities

            # range-reduce th -> rs in [-pi, pi]:  rs = th - 2pi*rint(th/2pi)
            q = t3("q", 2)
            qi = tp.tile([P, CH, 2], I32, tag="qi")
            qr = t3("qr", 2)
            rs = t3("rs", 2)
            nc.vector.tensor_scalar_mul(q[:], th, 1.0 / TWO_PI)
            nc.vector.tensor_copy(qi[:], q[:])
            nc.vector.tensor_copy(qr[:], qi[:])
            nc.vector.scalar_tensor_tensor(
                rs[:], in0=qr[:], scalar=-TWO_PI, in1=th,
                op0=ALU.mult, op1=ALU.add,
            )
            # range-reduce th + pi/2 -> rc (for cos)
            qc = t3("qc", 2)
            qci = tp.tile([P, CH, 2], I32, tag="qci")
            qcr = t3("qcr", 2)
            rc = t3("rc", 2)
            nc.vector.tensor_scalar(
                qc[:], th, 1.0 / TWO_PI, 0.25, op0=ALU.mult, op1=ALU.add
            )
            nc.vector.tensor_copy(qci[:], qc[:])
            nc.vector.tensor_copy(qcr[:], qci[:])
            nc.vector.scalar_tensor_tensor(
                rc[:], in0=qcr[:], scalar=-TWO_PI, in1=th,
                op0=ALU.mult, op1=ALU.add,
            )
            nc.vector.tensor_scalar_add(rc[:], rc[:], HALF_PI)

            sn = t3("sn", 2)
            cs = t3("cs", 2)
            nc.scalar.activation(sn[:], rs[:], AF.Sin)
            nc.scalar.activation(cs[:], rc[:], AF.Sin)

            s1, s2 = sn[:, :, 0:1], sn[:, :, 1:2]
            c1, c2 = cs[:, :, 0:1], cs[:, :, 1:2]
            w1v, w2v = wv_[:, :, 0:1], wv_[:, :, 1:2]

            px = t3("px")
            nc.vector.tensor_add(px[:], c1, c2)
            nc.vector.tensor_scalar_mul(px[:], px[:], 3.0)
            py = t3("py")
            nc.vector.tensor_add(py[:], s1, s2)
            nc.vector.tensor_scalar(py[:], py[:], 3.0, -7.0, op0=ALU.mult, op1=ALU.add)

            s1w = t3("s1w")
            nc.vector.tensor_mul(s1w[:], s1, w1v)
            s2w = t3("s2w")
            nc.vector.tensor_mul(s2w[:], s2, w2v)
            vx = t3("vx")
            nc.vector.tensor_add(vx[:], s1w[:], s2w[:])
            nc.vector.tensor_scalar_mul(vx[:], vx[:], -3.0)
            c1w = t3("c1w")
            nc.vector.tensor_mul(c1w[:], c1, w1v)
            c2w = t3("c2w")
            nc.vector.tensor_mul(c2w[:], c2, w2v)
            vy = t3("vy")
            nc.vector.tensor_add(vy[:], c1w[:], c2w[:])
            nc.vector.tensor_scalar_mul(vy[:], vy[:], 3.0)

            pxx = t3("pxx")
            nc.vector.tensor_mul(pxx[:], px[:], px[:])
            pyy = t3("pyy")
            nc.vector.tensor_mul(pyy[:], py[:], py[:])
            # barrier scaled by 16 = alpha*beta scale (4*sigmoid each)
            barrier = t3("barrier")
            nc.vector.tensor_add(barrier[:], pxx[:], pyy[:])
            nc.vector.tensor_scalar(
                barrier[:], barrier[:], 16.0, -256.0, op0=ALU.mult, op1=ALU.add
            )

            pv1 = t3("pv1")
            nc.vector.tensor_mul(pv1[:], px[:], vx[:])
            pv2 = t3("pv2")
            nc.vector.tensor_mul(pv2[:], py[:], vy[:])
            b_dot = t3("b_dot")
            nc.vector.tensor_add(b_dot[:], pv1[:], pv2[:])
            # 2 (from b_dot) * 4 (alpha+beta sigmoid scale)
            nc.vector.tensor_scalar_mul(b_dot[:], b_dot[:], 8.0)

            w1sq = t3("w1sq")
            nc.vector.tensor_mul(w1sq[:], w1v, w1v)
            w2sq = t3("w2sq")
            nc.vector.tensor_mul(w2sq[:], w2v, w2v)
            ca = t3("ca")
            nc.vector.tensor_mul(ca[:], c1, w1sq[:])
            cb = t3("cb")
            nc.vector.tensor_mul(cb[:], c2, w2sq[:])
            nc.vector.tensor_add(ca[:], ca[:], cb[:])   # c1*w1^2 + c2*w2^2
            sa = t3("sa")
            nc.vector.tensor_mul(sa[:], s1, w1sq[:])
            sb = t3("sb")
            nc.vector.tensor_mul(sb[:], s2, w2sq[:])
            nc.vector.tensor_add(sa[:], sa[:], sb[:])   # s1*w1^2 + s2*w2^2

            vxx = t3("vxx")
            nc.vector.tensor_mul(vxx[:], vx[:], vx[:])
            vyy = t3("vyy")
            nc.vector.tensor_mul(vyy[:], vy[:], vy[:])
            vsum = t3("vsum")
            nc.vector.tensor_add(vsum[:], vxx[:], vyy[:])
            nc.vector.tensor_scalar_mul(vsum[:], vsum[:], 2.0)  # 2vx^2+2vy^2

            pca = t3("pca")
            nc.vector.tensor_mul(pca[:], px[:], ca[:])
            psa = t3("psa")
            nc.vector.tensor_mul(psa[:], py[:], sa[:])
            nc.vector.tensor_add(pca[:], pca[:], psa[:])
            lf2b = t3("lf2b")
            nc.vector.scalar_tensor_tensor(
                lf2b[:], in0=pca[:], scalar=-6.0, in1=vsum[:],
                op0=ALU.mult, op1=ALU.add,
            )  # Lf2b = 2(vx^2+vy^2) - 6*(px*ca + py*sa)

            g1 = t3("g1")
            m1 = t3("m1")
            nc.vector.tensor_mul(m1[:], py[:], c1)
            m2 = t3("m2")
            nc.vector.tensor_mul(m2[:], px[:], s1)
            nc.vector.tensor_sub(g1[:], m1[:], m2[:])
            nc.vector.tensor_scalar_mul(g1[:], g1[:], 6.0)
            g2 = t3("g2")
            nc.vector.tensor_mul(m1[:], py[:], c2)
            nc.vector.tensor_mul(m2[:], px[:], s2)
            nc.vector.tensor_sub(g2[:], m1[:], m2[:])
            nc.vector.tensor_scalar_mul(g2[:], g2[:], 6.0)

            gdot = t3("gdot")
            g1sq = t3("g1sq")
            nc.vector.tensor_mul(g1sq[:], g1[:], g1[:])
            g2sq = t3("g2sq")
            nc.vector.tensor_mul(g2sq[:], g2[:], g2[:])
            nc.vector.tensor_add(gdot[:], g1sq[:], g2sq[:])
            igdot = t3("igdot")
            nc.vector.reciprocal(igdot[:], gdot[:])
            g12 = tp.tile([P, CH, 2], F32, tag="g12", name="g12")
            nc.vector.tensor_copy(g12[:, :, 0:1], g1[:])
            nc.vector.tensor_copy(g12[:, :, 1:2], g2[:])

            # ---- L3 ----
            h3 = hpool.tile([P, 16, B], h3dt, tag="act", name="h3")
            sc3 = esc3t[:]
            mlp_layer(w3, 16, 16, 2048, h2, 0, b3t, ev_h(h3, b3t, 0, sc3),
                      dr="l3" in fp8_layers)

            h4 = hpool.tile([P, 16, B], BF16, tag="act", name="h4")
            l4dr = "l4" in fp8_layers
            w4dt = FP8 if l4dr else BF16

            # preload all of W41/W42 into SBUF during the L3 phase so the
            # per-half L4 loops below don't re-stream HBM weights.
            w41c = w4pool.tile([P, 8, 8, P], w4dt, tag="w41", name="w41c")
            nc.sync.dma_start(w41c[:], w41)
            w42c = w4pool.tile([P, 8, 8, P], w4dt, tag="w42", name="w42c")
            nc.sync.dma_start(w42c[:], w42)

            # ---- L5 eviction into the combined head tile (per n-chunk) ----
            def ev_51(n, ps):
                if n == 3:
                    # last quarter: DVE is busy with the QP chains; ACT is
                    # free and this eviction gates the final transposes
                    nc.scalar.activation(
                        x5cat[:20, n * NF : (n + 1) * NF], ps, AF.Identity,
                        bias=b51t[:],
                    )
                else:
                    nc.vector.tensor_scalar_add(
                        x5cat[:20, n * NF : (n + 1) * NF], ps, b51t[:]
                    )

            def ev_52(n, ps):
                nc.scalar.activation(
                    x5cat[32:43, n * NF : (n + 1) * NF], ps, AF.Sigmoid,
                    bias=b52t[:],
                )

            def l5_quarter(wc, kin_base, mp, n, evict):
                # psumB: the main-stream pool is idle once L4's last psum
                # retires, and using it keeps psumA free for the transposes
                ps = psumB.tile([P, 2 * NF], F32, tag="mm", name="l5ps")
                for k in range(8):
                    nc.tensor.matmul(
                        ps[:mp, :NF],
                        wc[:, k, :],
                        h4[:, kin_base + k, n * NF : (n + 1) * NF],
                        start=(k == 0),
                        stop=(k == 7),
                    )
                evict(n, ps[:mp, :NF])

            # ---- tail part 2 tiles (shared across quarters) ----
            x5t = tp.tile([P, CH, 43], F32, tag="x5t")
            apb = t3("apb", 10)
            ab = t3("ab", 10)
            hv = t3("hv", 10)
            hv2 = t3("hv2", 10)
            refg = tp.tile([P, CH, 10, 2], F32, tag="refg", name="refg")
            viol = t3("viol", 10)
            lam = t3("lam", 10)
            wlam = t3("wlam", 10)
            S = t3("S")
            wref = t3("wref", 20)
            rbxy = t3("rbxy", 2)
            rtxy = t3("rtxy", 2)
            ot = t3("ot", 2)
            wv20 = const.tile([P, 20], F32, tag="wv20")

            def qp_tail(c0, c1, eng):
                W = c1 - c0
                BW10 = [P, W, 10]
                cs_ = (slice(None), slice(c0, c1))
                alpha = x5t[:, c0:c1, 32:33]
                betas = x5t[:, c0:c1, 33:43]
                eng.tensor_add(apb[*cs_], betas, alpha.to_broadcast(BW10))
                eng.tensor_mul(ab[*cs_], betas, alpha.to_broadcast(BW10))
                eng.tensor_mul(
                    hv[*cs_], apb[*cs_], b_dot[:, c0:c1, :].to_broadcast(BW10)
                )
                eng.tensor_mul(
                    hv2[*cs_], ab[*cs_], barrier[:, c0:c1, :].to_broadcast(BW10)
                )
                eng.tensor_add(hv[*cs_], hv[*cs_], hv2[*cs_])
                eng.tensor_add(
                    hv[*cs_], hv[*cs_], lf2b[:, c0:c1, :].to_broadcast(BW10)
                )
                eng.tensor_mul(
                    refg[:, c0:c1],
                    x5t[:, c0:c1, 0:20].rearrange("p c (h two) -> p c h two", two=2),
                    g12[:, c0:c1, None, :].to_broadcast([P, W, 10, 2]),
                )
                nc.vector.reduce_sum(viol[*cs_, slice(None)], refg[:, c0:c1], axis=AX.X)
                eng.tensor_sub(viol[*cs_], viol[*cs_], hv[*cs_])
                eng.tensor_mul(
                    lam[*cs_], viol[*cs_], igdot[:, c0:c1, :].to_broadcast(BW10)
                )
                eng.tensor_scalar_max(lam[*cs_], lam[*cs_], 0.0)
                eng.tensor_mul(
                    wlam[*cs_], lam[*cs_], wv20[:, None, 0::2].to_broadcast(BW10)
                )
                nc.vector.reduce_sum(S[*cs_, 0], wlam[*cs_], axis=AX.X)
                eng.tensor_mul(
                    wref[*cs_], x5t[:, c0:c1, 0:20],
                    wv20[:, None, :].to_broadcast([P, W, 20]),
                )
                nc.vector.reduce_sum(
                    rbxy[*cs_, slice(None)],
                    wref[*cs_].rearrange("p c (h two) -> p c two h", two=2),
                    axis=AX.X,
                )
                eng.tensor_mul(
                    rtxy[*cs_], g12[:, c0:c1], S[:, c0:c1, :].to_broadcast([P, W, 2])
                )
                eng.tensor_sub(rtxy[*cs_], rtxy[*cs_], rbxy[*cs_])
                eng.tensor_scalar(
                    ot[*cs_, 0], rtxy[*cs_, 0], mlt[:, 0:1], islt[:, 0:1],
                    op0=ALU.subtract, op1=ALU.mult,
                )
                eng.tensor_scalar(
                    ot[*cs_, 1], rtxy[*cs_, 1], mlt[:, 1:2], islt[:, 1:2],
                    op0=ALU.subtract, op1=ALU.mult,
                )

            # ---- L4 in halves; L5/transpose/QP in quarters.  Quarter q's
            # vector tail hides under quarter q+1's matmul stream; QP chains
            # alternate DVE / GpSimd so consecutive quarters overlap.
            def tail_quarter(n):
                for c in range(4 * n, 4 * n + 4):
                    pt = psumA.tile([P, 2 * NF], F32, tag="mm", name="pt")
                    nc.tensor.transpose(
                        pt[:, :64], x5cat[:, c * P : (c + 1) * P], ident[:64, :64]
                    )
                    if c % 2 == 0:
                        nc.vector.tensor_copy(x5t[:, c, :], pt[:, :43])
                    else:
                        nc.scalar.copy(x5t[:, c, :], pt[:, :43])
                eng = nc.gpsimd if n in (1, 2) else nc.vector
                qp_tail(4 * n, 4 * (n + 1), eng)
                nc.sync.dma_start(out[:, 4 * n : 4 * (n + 1)], ot[:, 4 * n : 4 * (n + 1)])

            for hf in range(2):
                mlp_layer(None, 8, 8, 1024, h3, 0, b41t,
                          ev_h(h4, b41t, 0, 1.0, alt=True),
                          prefetched={m: w41c[:, m] for m in range(8)},
                          dr=l4dr, groups=(hf,))
                # the x51 head reads only h4[0:8] (the W41 branch): emit it
                # here so it streams while the W42 GEMM's evictions land
                l5_quarter(w51c, 0, 20, 2 * hf, ev_51)
                l5_quarter(w51c, 0, 20, 2 * hf + 1, ev_51)
                mlp_layer(None, 8, 8, 1024, h3, 8, b42t,
                          ev_h(h4, b42t, 8, 1.0, alt=True),
                          prefetched={m: w42c[:, m] for m in range(8)},
                          dr=l4dr, groups=(hf,))
                if hf == 0:
                    # wv broadcast to all partitions (one PE pass)
                    pwv = psumA.tile([P, 2 * NF], F32, tag="mm", name="pwv")
                    nc.tensor.matmul(
                        pwv[:, :32], onesp[:], wvp[:], start=True, stop=True
                    )
                    nc.vector.tensor_copy(wv20[:], pwv[:, :20])
                for n in (2 * hf, 2 * hf + 1):
                    l5_quarter(w52c, 8, 11, n, ev_52)
                    tail_quarter(n)

    nc.compile()
    return nc


def configure(fp8_layers):
    """Select fp8 layers; must be called before the first kernel() call."""
    global FP8_LAYERS, _CACHED_NC
    if frozenset(fp8_layers) != FP8_LAYERS:
        FP8_LAYERS = frozenset(fp8_layers)
        _CACHED_NC = None


def _get_nc():
    global _CACHED_NC
    if _CACHED_NC is None:
        _CACHED_NC = _build(FP8_LAYERS)
    return _CACHED_NC


def _bf16(a):
    return np.ascontiguousarray(a.astype(ml_dtypes.bfloat16))


def _f32(a):
    return np.ascontiguousarray(np.asarray(a, dtype=np.float32))


def _e4(a):
    dt = mybir.dt.np(mybir.dt.float8e4)
    return np.ascontiguousarray(a.astype(dt))


def _prep_inputs(inputs):
    x = _f32(inputs["x"])
    mean = _f32(inputs["mean"])
    std = _f32(inputs["std"])
    mean_label = _f32(inputs["mean_label"])
    std_label = _f32(inputs["std_label"])
    wt = _f32(inputs["wt"])
    W1, b1 = _f32(inputs["W1"]), _f32(inputs["b1"])
    W2, b2 = _f32(inputs["W2"]), _f32(inputs["b2"])
    W31, b31 = _f32(inputs["W31"]), _f32(inputs["b31"])
    W32, b32 = _f32(inputs["W32"]), _f32(inputs["b32"])
    W41, b41 = _f32(inputs["W41"]), _f32(inputs["b41"])
    W42, b42 = _f32(inputs["W42"]), _f32(inputs["b42"])
    W51, b51 = _f32(inputs["W51"]), _f32(inputs["b51"])
    W52, b52 = _f32(inputs["W52"]), _f32(inputs["b52"])

    def pack3(wT, KT):  # (K, M) -> (128, KT, M)
        K, M = wT.shape
        return _bf16(wT.reshape(KT, P, M).transpose(1, 0, 2))

    W3T = np.concatenate([W31.T, W32.T], axis=1)  # (2048, 2048)
    b3 = np.concatenate([b31, b32])

    # fp8 calibration: static activation scales from a row subsample
    # (4.7x headroom to the e4m3 max), per-tensor weight scales.
    sa1 = sa2 = sa3 = 1.0
    sw2 = sw3 = sw4 = 1.0
    if FP8_LAYERS:
        xs = x[:512]
        h1s = np.maximum(xs @ W1.T + b1, 0.0)
        if "l2" in FP8_LAYERS:
            sa1 = 96.0 / max(float(np.abs(h1s).max()), 1e-30)
            sw2 = 192.0 / max(float(np.abs(W2).max()), 1e-30)
        if "l3" in FP8_LAYERS or "l4" in FP8_LAYERS:
            h2s = np.maximum(h1s @ W2.T + b2, 0.0)
            if "l3" in FP8_LAYERS:
                sa2 = 96.0 / max(float(np.abs(h2s).max()), 1e-30)
                sw3 = 192.0 / max(float(np.abs(W3T).max()), 1e-30)
            if "l4" in FP8_LAYERS:
                h3s = np.maximum(h2s @ W3T + b3, 0.0)
                sa3 = 96.0 / max(float(np.abs(h3s).max()), 1e-30)
                sw4 = 192.0 / max(
                    float(max(np.abs(W41).max(), np.abs(W42).max())), 1e-30
                )
    esc2 = sa2 / (sw2 * sa1)
    esc3 = sa3 / (sw3 * sa2)
    esc4 = 1.0 / (sw4 * sa3)

    def wpack(wT, KT, MT, sw, fp8):
        packed = wT.reshape(KT, P, MT, P).transpose(1, 2, 0, 3)
        if fp8:
            return _e4(packed * sw)
        return _bf16(packed)

    w14 = np.concatenate([W1.T] * 4, axis=0)  # (16, 2048), rows 4i+j = W1.T[j]
    shared = {
        "w14": _bf16(w14 * sa1),
        "w2": wpack(W2.T, 16, 16, sw2, "l2" in FP8_LAYERS),
        "w3": wpack(W3T, 16, 16, sw3, "l3" in FP8_LAYERS),
        "w41": wpack(W41.T, 8, 8, sw4, "l4" in FP8_LAYERS),
        "w42": wpack(W42.T, 8, 8, sw4, "l4" in FP8_LAYERS),
        "w51": pack3(W51.T * esc4, 8),
        "w52": pack3(W52.T * esc4, 8),
        "esc2": _f32(np.full((P, 1), esc2)),
        "esc3": _f32(np.full((P, 1), esc3)),
        "esc4": _f32(np.full((P, 1), esc4)),
        "b1": _f32(b1.reshape(16, P).T * sa1),
        "b2": _f32(b2.reshape(16, P).T * sa2),
        "b3": _f32(b3.reshape(16, P).T * sa3),
        "b41": _f32(b41.reshape(8, P).T / esc4),
        "b42": _f32(b42.reshape(8, P).T / esc4),
        "b51": b51,
        "b52": b52,
        "stdb": _f32(np.tile(std[None, :], (P, 1))),
        "meanb": _f32(np.tile(mean[None, :], (P, 1))),
        "mlb": _f32(np.tile(mean_label[None, :], (P, 1))),
        "islb": _f32(np.tile((1.0 / std_label)[None, :], (P, 1))),
        "wtv": wt,
    }

    in_maps = []
    for i in range(N_CORES):
        xs = x[i * B : (i + 1) * B]  # (2048, 4)
        m = dict(shared)
        m["xt4"] = _bf16(np.concatenate([xs.T] * 4, axis=0))  # (16, B)
        m["xn"] = _f32(xs.reshape(CH, P, 4).transpose(1, 0, 2))
        in_maps.append(m)
    return in_maps


def kernel_core(inputs, trace=False):
    nc = _get_nc()
    in_maps = _prep_inputs(inputs)
    res = run_bass_kernel_spmd(
        nc, in_maps, core_ids=list(range(N_CORES)), trace=trace
    )
    shards = []
    for i in range(N_CORES):
        o = res.results[i]["out"]  # (128, 16, 2)
        shards.append(o.transpose(1, 0, 2).reshape(B, 2))
    return np.concatenate(shards, axis=0).astype(np.float32), res


def kernel(**inputs):
    out, _ = kernel_core(inputs)
    return out

